# revision 54
# baseline (speedup 1.0000x reference)
"""KANConvTranspose2d forward on 8 Trainium2 NeuronCores.

Sharding: row-parallel over in_features (2304/8 = 288 per core).
Host pre-folds spline_scaler into spline_weight and packs base+spline
weights into one fp16 matrix [2592, OUT_F] per core (rows = 27 K-chunks
of (96-feature chunk, term), columns permuted into two per-channel
parts).  Each core evaluates its b-splines via the truncated-power
(relu^3) form of the uniform-knot B-spline, then runs weights-stationary
matmuls (lhsT = weight tile [K<=96, 128], moving = bases [K, 64])
accumulating o-major [OUT_F, B] partials in PSUM.  The output columns
are split 2:1 so the first part's ReduceScatter hides under the second
part's weight stream; after each RS, core c holds a slice of its own
output channel, which PE transposes flip to [n, o] for the fold.
"""

import numpy as np

import concourse.bacc as bacc
import concourse.bass as bass
import concourse.mybir as mybir
import concourse.tile as tile
from concourse.bass_utils import run_bass_kernel_spmd

# module constants
CIN, COUT = 16, 8
HIN = WIN = 8
KK, ST, PD = 3, 2, 1
GRID_SIZE, SPLINE_ORDER = 5, 3
HOUT = WOUT = 16
OH_IN = OW_IN = 4
OH_OUT = OW_OUT = 8
IN_F = CIN * KK * KK * OH_IN * OW_IN        # 2304
OUT_F = COUT * KK * KK * OH_OUT * OW_OUT    # 4608
B = 64
NCORE = 8
IC = IN_F // NCORE                          # 288 in_features per core
OSH = OUT_F // NCORE                        # 576 out_features per core
NS = GRID_SIZE + SPLINE_ORDER               # 8 spline bases per feature
NG = GRID_SIZE + 2 * SPLINE_ORDER + 1       # 12 grid knots per feature
NT = NS + 1                                 # 9 terms (base + 8 spline)

# per-core contraction chunking: 288 = 3 x 96 (uniform chunks keep the
# weight stream and K-chunk bookkeeping simple)
PC = 96
CHUNKS = [(0, PC), (PC, PC), (2 * PC, PC)]
KROWS = IC * NT                             # 2592 packed weight rows per core
NM = OUT_F // 128                           # 36 output chunks of 128
NBANK = (NM + 7) // 8                       # 36 chunks -> 5 psum banks (last half)

F32 = mybir.dt.float32
F16 = mybir.dt.float16

_CACHE = {}


def _build_bass(tscale, tbias):
    nc = bacc.Bacc("TRN2", target_bir_lowering=False, debug=False,
                   num_devices=NCORE)
    uT_d = nc.dram_tensor("uT", [IC, B], F32, kind="ExternalInput")
    io_d = nc.dram_tensor("iota", [128, NG], F32, kind="ExternalInput")
    w_d = nc.dram_tensor("wpk", [KROWS, OUT_F], F16, kind="ExternalInput")
    eye_d = nc.dram_tensor("eye", [96, 96], F16, kind="ExternalInput")
    y_d = nc.dram_tensor("y", [B, HOUT * WOUT], F32, kind="ExternalOutput")
    # collective bounce buffers (o-major partials), one per column part
    P_d = [nc.dram_tensor("partial0", [3072, B], F16),
           nc.dram_tensor("partial1", [1536, B], F16)]
    R_d = [nc.dram_tensor("reduced0", [384, B], F16),
           nc.dram_tensor("reduced1", [192, B], F16)]

    with tile.TileContext(nc) as tc:
        with (
            tc.tile_pool(name="const", bufs=1) as cpool,
            tc.tile_pool(name="btmp", bufs=1) as bpool,
            tc.tile_pool(name="win", bufs=4) as wpool,
            tc.tile_pool(name="epi", bufs=1) as epool,
            tc.tile_pool(name="psum", bufs=1, space="PSUM") as pspool,
            tc.tile_pool(name="psum2", bufs=1, space="PSUM") as p2pool,
        ):
            # ---------------- phase 1: b-splines per i-chunk ----------------
            # Uniform grid -> truncated-power form: with t = (u - g0)/h and
            # r_j = relu(t - j)^3, bases[s] = (r_s - 4r_{s+1} + 6r_{s+2}
            # - 4r_{s+3} + r_{s+4}) / 6.  Matches de Boor exactly on a
            # uniform knot vector.
            io_t = cpool.tile([PC, NG], F32, tag="iota")
            u_t = cpool.tile([PC, 3, B], F32, tag="u")
            # head of the SP queue: u beats the weight stream to the DMA
            # engines (the spline chain gates the first K-chunks)
            nc.sync.dma_start(
                out=u_t[:], in_=uT_d.rearrange("(c p) n -> p c n", p=PC))
            nc.scalar.dma_start(out=io_t[:], in_=io_d[0:PC, :])
            si_all = cpool.tile([PC, 3, B], F16, tag="si")
            nc.scalar.activation(si_all[:], u_t[:],
                                 mybir.ActivationFunctionType.Silu)
            # host bakes the grid offset into iota: io = j - (-g0/h), so
            # t - j = u*(1/h) - io in a single fused DVE op
            bases_f16 = []
            for ci in range(3):
                cb = bpool.tile([PC, NG, B], F32, tag="cb")
                nc.vector.scalar_tensor_tensor(
                    out=cb[:],
                    in0=u_t[:, ci, :].unsqueeze(1).broadcast_to(
                        [PC, NG, B]),
                    scalar=tscale,
                    in1=io_t[:].unsqueeze(2).broadcast_to([PC, NG, B]),
                    op0=mybir.AluOpType.mult,
                    op1=mybir.AluOpType.subtract)
                nc.vector.tensor_scalar_max(cb[:], cb[:], 0.0)
                sq = bpool.tile([PC, NG, B], F32, tag="sq")
                nc.vector.tensor_tensor(out=sq[:], in0=cb[:], in1=cb[:],
                                        op=mybir.AluOpType.mult)
                nc.vector.tensor_tensor(out=cb[:], in0=sq[:], in1=cb[:],
                                        op=mybir.AluOpType.mult)
                a1 = bpool.tile([PC, NS, B], F32, tag="a1")
                nc.vector.tensor_tensor(out=a1[:], in0=cb[:, 0:NS, :],
                                        in1=cb[:, 4:4 + NS, :],
                                        op=mybir.AluOpType.add)
                a2 = bpool.tile([PC, NS, B], F32, tag="a2")
                nc.vector.tensor_tensor(out=a2[:], in0=cb[:, 1:1 + NS, :],
                                        in1=cb[:, 3:3 + NS, :],
                                        op=mybir.AluOpType.add)
                nc.vector.scalar_tensor_tensor(
                    out=a2[:], in0=a2[:], scalar=-4.0, in1=a1[:],
                    op0=mybir.AluOpType.mult, op1=mybir.AluOpType.add)
                nc.vector.scalar_tensor_tensor(
                    out=a2[:], in0=cb[:, 2:2 + NS, :], scalar=6.0, in1=a2[:],
                    op0=mybir.AluOpType.mult, op1=mybir.AluOpType.add)
                bb = cpool.tile([PC, NS, B], F16, tag=f"bb{ci}")
                nc.vector.tensor_scalar_mul(bb[:], a2[:], 1.0 / 6.0)
                bases_f16.append(bb)

            # ---------------- phase 2: weight stream + matmul ----------------
            # Columns are permuted on the host into two parts: part 0 holds
            # o_local [0, 384) and part 1 holds [384, 576) of every output
            # channel.  Part 0 is streamed (27 K-chunks x cols 0:3072) and
            # reduced while part 1 streams, hiding the first ReduceScatter.
            # psum: part 0 -> banks 0-2 (24 M-chunks), part 1 -> banks 3-4.
            ps = [pspool.tile([128, 512], F32, tag=f"ps{b}", name=f"ps{b}")
                  for b in range(5)]
            # rhs AP per K-chunk (t=0 silu, else spline t-1), 96 rows each,
            # matching host weight row order (ci, t, i)
            kchunks = []
            for ci in range(3):
                kchunks.append(lambda c=ci: si_all[:, c, :])
                for s in range(NS):
                    kchunks.append(lambda c=ci, s=s: bases_f16[c][:, s, :])

            eye = epool.tile([96, 96], F16, tag="eye")
            nc.gpsimd.dma_start(out=eye[:], in_=eye_d[:])
            o_sb = epool.tile([B, HOUT + 2, WOUT + 2], F32, tag="osb")
            nc.vector.memset(o_sb[:], 0.0)
            rtb = [p2pool.tile([B, 1024], F16, tag=f"rtb{i}",
                                name=f"rtb{i}") for i in range(2)]

            # (column start, n M-chunks, psum bank base); weight DMAs move
            # 3 K-chunks per instruction on alternating queues, all through
            # ONE tile ring so part 1's stream queues behind part 0's
            PARTS = [(0, 24, 0), (3072, 12, 3)]
            for h, (c0, hm, bb_) in enumerate(PARTS):
                ncol = hm * 128
                # part 1 ends with single-chunk groups so the final PE
                # burst (and thus the last ReduceScatter) starts sooner
                grps = ([(g, 3) for g in range(0, 27, 3)] if h == 0 else
                        [(g, 3) for g in range(0, 24, 3)] +
                        [(24, 1), (25, 1), (26, 1)])
                for gi, (q0, gn) in enumerate(grps):
                    w_t = wpool.tile([PC, 3, 3072], F16, tag="w")
                    eng = nc.sync if gi % 2 == 0 else nc.scalar
                    eng.dma_start(
                        out=w_t[:, 0:gn, 0:ncol],
                        in_=w_d[q0 * PC:(q0 + gn) * PC,
                                c0:c0 + ncol].rearrange(
                            "(c p) o -> p c o", p=PC))
                    for j in range(gn):
                        q = q0 + j
                        rhs = kchunks[q]()
                        for m in range(hm):
                            bank, col = divmod(m, 8)
                            # start/stop once per BANK: start=True lazily
                            # zeroes the whole 2KB zero region, so only the
                            # first matmul into a bank may set it (later
                            # slices overwrite their still-pending bytes),
                            # and only the last write may stop the group
                            start = q == 0 and col == 0
                            stop = (q == 26
                                    and (col == 7 or m == hm - 1))
                            nc.tensor.matmul(
                                ps[bb_ + bank][:, col * B:(col + 1) * B],
                                w_t[:, j, m * 128:(m + 1) * 128], rhs,
                                start=start, stop=stop)

                # ---- per-part epilogue: psum -> P_h -> RS -> fold ----
                for b in range((hm + 7) // 8):
                    ncols = min(8, hm - b * 8)
                    yb = epool.tile([128, ncols, B], F16, tag=f"yb{h}{b}")
                    src = ps[bb_ + b][:, 0:ncols * B]
                    dst = yb[:].rearrange("p m n -> p (m n)")
                    if b % 2 == 0:
                        nc.vector.tensor_copy(out=dst, in_=src)
                    else:
                        nc.scalar.activation(
                            dst, src, mybir.ActivationFunctionType.Copy)
                    # ACT queue (HWDGE): by the time ACT reaches these it
                    # has issued all its weight groups, and the faster
                    # issue path shortens the copy -> RS critical chain
                    nc.scalar.dma_start(
                        out=P_d[h][b * 1024:b * 1024 + ncols * 128, :]
                        .rearrange("(m p) n -> p m n", p=128),
                        in_=yb[:])
                nc.gpsimd.collective_compute(
                    "ReduceScatter", mybir.AluOpType.add,
                    replica_groups=[list(range(NCORE))],
                    ins=[P_d[h][:]], outs=[R_d[h][:]])
                # r_in[p, k, n] = reduced[o_part = 96k + p, n]
                nk = hm * 16 // 96              # 96-row transpose blocks
                r_in = epool.tile([96, nk, B], F16, tag=f"rin{h}",
                                  name=f"rin{h}")
                # SP queue is drained of weight groups by the time the RS
                # completes; its HWDGE path beats Pool's SWDGE here
                nc.sync.dma_start(
                    out=r_in[:], in_=R_d[h].rearrange("(k p) n -> p k n",
                                                      p=96))
                # transpose each [96, B] block, then scatter-add pieces:
                # channel-local o = c0/8 + 96k + col; o = kk*64 + (oh*8+ow)
                for k in range(nk):
                    rT = rtb[h][:, k * 96:(k + 1) * 96]
                    nc.tensor.transpose(rT, r_in[:, k, :], eye[:])
                    obase = c0 // 8 + 96 * k
                    a = obase
                    while a < obase + 96:
                        bnd = min((a // B + 1) * B, obase + 96)
                        kk_, s0 = divmod(a, B)
                        kh, kw = divmod(kk_, KK)
                        nrow = (bnd - a) // OW_OUT
                        oh0 = s0 // OW_OUT
                        dst = o_sb[:, kh + 2 * oh0:kh + 2 * (oh0 + nrow):2,
                                   kw:kw + 2 * OW_OUT:2]
                        nc.vector.tensor_tensor(
                            out=dst, in0=dst,
                            in1=rT[:, a - obase:bnd - obase].rearrange(
                                "p (a b) -> p a b", b=OW_OUT),
                            op=mybir.AluOpType.add)
                        a = bnd
            nc.sync.dma_start(out=y_d[:],
                              in_=o_sb[:, 1:1 + HOUT, 1:1 + WOUT])

    nc.compile()
    return nc


def _unfold(x):
    xp = np.pad(x, ((0, 0), (0, 0), (PD, PD), (PD, PD)))
    pats = np.stack(
        [xp[:, :, i:i + (OH_IN - 1) * ST + 1:ST, j:j + (OW_IN - 1) * ST + 1:ST]
         for i in range(KK) for j in range(KK)], axis=2)
    return pats.reshape(B, CIN * KK * KK, OH_IN * OW_IN).reshape(B, IN_F)


def kernel(x, base_weight, spline_weight, spline_scaler, grid):
    grid = np.asarray(grid, np.float32)
    h = float(grid[0, 1] - grid[0, 0])
    g0 = float(grid[0, 0])
    if "nc" not in _CACHE:
        _CACHE["nc"] = _build_bass(1.0 / h, -g0 / h)
    nc = _CACHE["nc"]

    uT = np.ascontiguousarray(_unfold(np.asarray(x, np.float32)).T)  # [IN_F,B]
    eye = np.eye(96, dtype=np.float16)
    # iota carries the grid offset: io = j + g0/h, so t - j = u/h - io
    iota = np.broadcast_to(
        np.arange(NG, dtype=np.float32) + g0 / h, (128, NG)).copy()

    # fold scaler into spline weights; pack [IN_F, 9, OUT_F] fp16
    bw = np.asarray(base_weight, np.float32)
    sw = np.asarray(spline_weight, np.float32)
    sc = np.asarray(spline_scaler, np.float32)
    scaled = sw * sc[:, :, None]                      # [OUT_F, IN_F, 8]
    wf = np.empty((IN_F, NT, OUT_F), np.float16)
    wf[:, 0, :] = bw.T
    wf[:, 1:, :] = scaled.transpose(1, 2, 0)
    # permute columns into parts: part 0 = o_local [0, 384), part 1 =
    # [384, 576) of each output channel, matching the split ReduceScatter
    perm = np.concatenate(
        [np.arange(lo, hi) + c * OSH
         for lo, hi in ((0, 384), (384, OSH)) for c in range(COUT)])
    wf = wf[:, :, perm]

    in_maps = []
    for c in range(NCORE):
        r0, r1 = c * IC, (c + 1) * IC
        blk = wf[r0:r1]                               # [288, 9, OUT_F]
        # rows (ci, t, i_local) in 96-feature chunks
        parts = [np.ascontiguousarray(
            blk[off:off + p].transpose(1, 0, 2)).reshape(-1, OUT_F)
            for off, p in CHUNKS]
        in_maps.append({
            "uT": np.ascontiguousarray(uT[r0:r1]),
            "iota": iota,
            "wpk": np.ascontiguousarray(np.concatenate(parts, axis=0)),
            "eye": eye,
        })

    res = run_bass_kernel_spmd(nc, in_maps, list(range(NCORE)))
    out = np.stack(
        [res.results[c]["y"].reshape(B, HOUT, WOUT) for c in range(NCORE)],
        axis=1)
    return np.ascontiguousarray(out.astype(np.float32))
